# revision 13
# baseline (speedup 1.0000x reference)
"""CrossAttentionPool forward on 8 TRN2 NeuronCores.

Reference computation (per batch b):
    q = lines[b] @ w_q.T ; k = videos[b] @ w_k.T
    scores = (q @ k.T) * D**-0.5, masked where video_mask==0
    out = softmax(scores, axis=-1) @ videos[b]

Strategy (data-parallel over batch, 4 batches/core):
    scores = lines @ W @ videos^T with W = (w_q.T @ w_k) * scale folded on host.
    All operands bf16; output stored bf16 and widened to fp32 on host.

    Device pipeline (memory-roofline ~9.0MB @ ~390GB/s/core):
      - PE warmup matmuls ramp the DVFS p-state (0.65/1.2/2.4 GHz; max only
        after 3us CONTINUOUS busy) while inputs stream; the compute order
        (u-pair0 -> batches 0,1 -> u-pair1 -> batches 2,3) keeps the PE from
        ever idling on a late input, which would reset the clock to 1.2GHz.
      - Two input queues: Sync carries the u-critical stream (vT01, wl
        m-slices, vones, vT23, lT2); Scalar carries lines b0,b1,b3.
      - u[d,v] = sum_d' W[d,d'] videos[v,d']  (36 MMs/pair, N=256)
      - scores^T = u^T lines^T (6 MMs N=512/batch); exp is ONE 512-wide
        Scalar activation per batch with the mask as exp-bias (-50).
      - out rows + softmax denominator in the same matmuls via two ones
        columns appended to videos; per l-chunk: DVE reciprocal, scaled
        PSUM drains alternating Scalar/DVE (GpSimd cannot read PSUM),
        per-chunk contiguous bf16 stores issued from GpSimd.
    PSUM rings (1 bank per buf, 8 banks): st 2, pu 2, po1 2, po2 2.
"""
import numpy as np
from contextlib import ExitStack
import concourse.bacc as bacc
import concourse.tile as tile
from concourse import mybir
from concourse.bass_utils import run_bass_kernel_spmd

N_CORES = 8
B, L, V, D = 32, 512, 128, 768
BPC = B // N_CORES          # batches per core
KC = D // 128               # 6 contraction chunks
LC = L // 128               # 4 line chunks
F32 = mybir.dt.float32
BF16 = mybir.dt.bfloat16
N_WARM = 9                  # warmup matmuls (ap=512) to ramp the PE clock


def _u_pair(nc, pair, pp_u, vs, us, wl_r, dr, ndr, hooks=None):
    """u = W @ videos^T for one batch pair (N=256).

    hooks[m]() runs right after m's drain — used to issue the lines DMAs
    from Scalar only once the u stream is underway, so the critical Q-A
    stream has the full bandwidth at kernel start.
    """
    for m in range(KC):
        pu = pp_u.tile([128, 256], F32, tag="pu")
        for c in range(KC):
            nc.tensor.matmul(pu[:], wl_r[:, m, c], vs[pair][:, c],
                             start=(c == 0), stop=(c == KC - 1))
        dr[ndr % 2](us[pair][:, m], pu[:])
        ndr += 1
        if hooks and m in hooks:
            hooks[m]()
    return ndr


def _scores(nc, b, us, lT, pp_st, etpool, maskb):
    psT = pp_st.tile([128, 512], F32, tag="st")
    ub = us[b // 2]
    for m in range(KC):
        nc.tensor.matmul(psT[:], ub[:, m, (b % 2) * V:(b % 2 + 1) * V],
                         lT[b][:, m, :], start=(m == 0), stop=(m == KC - 1))
    eT = etpool.tile([128, 512], BF16)
    nc.scalar.activation(eT[:], psT[:], mybir.ActivationFunctionType.Exp,
                         bias=maskb[:, b:b + 1])
    return eT


def _out_batch(nc, b, eT, vbr, pp_o1, pp_o2, rpool, outpool, out_d):
    """out rows + denominators for one batch; drains split Scalar/DVE."""
    for i in range(LC):
        ech = eT[:, i * 128:(i + 1) * 128]
        po1 = pp_o1.tile([128, 512], F32, tag="po1")
        nc.tensor.matmul(po1[:], ech, vbr[:, b, 0:512], start=True, stop=True)
        po2 = pp_o2.tile([128, 258], F32, tag="po2")
        nc.tensor.matmul(po2[:], ech, vbr[:, b, 512:D + 2],
                         start=True, stop=True)
        rec = rpool.tile([128, 1], F32)
        nc.vector.reciprocal(rec[:], po2[:, 256:257])
        osb = outpool.tile([128, D], BF16)
        if i % 2 == 0:
            nc.scalar.mul(osb[:, 0:512], po1[:], rec[:])
            nc.vector.tensor_scalar_mul(osb[:, 512:D], po2[:, 0:256], rec[:])
        else:
            nc.vector.tensor_scalar_mul(osb[:, 0:512], po1[:], rec[:])
            nc.scalar.mul(osb[:, 512:D], po2[:, 0:256], rec[:])
        seng = nc.gpsimd if (b * LC + i) % 2 == 0 else nc.sync
        seng.dma_start(out_d[b, i * 128:(i + 1) * 128, :], osb[:])


def _body(tc, out_d, linesT_d, vT01_d, vT23_d, vones_d, maskb_d, wl_d):
    nc = tc.nc
    with ExitStack() as ctx:
        const = ctx.enter_context(tc.tile_pool(name="const", bufs=1))
        persist = ctx.enter_context(tc.tile_pool(name="persist", bufs=1))
        etpool = ctx.enter_context(tc.tile_pool(name="etp", bufs=2))
        outpool = ctx.enter_context(tc.tile_pool(name="osb", bufs=6))
        rpool = ctx.enter_context(tc.tile_pool(name="rp", bufs=4))

        pp_st = ctx.enter_context(tc.tile_pool(name="pp_st", bufs=2, space="PSUM"))

        # --- warmup operand (memset early on gpsimd; no DMA) ----------------
        wsrc = const.tile([128, 512], BF16)
        nc.gpsimd.memset(wsrc[:], 0.125)
        maskb = const.tile([128, BPC], F32)

        # --- input DMAs -----------------------------------------------------
        # Q-A (Sync): u-critical stream, strictly ordered, then vones, lT2
        vT01 = persist.tile([128, KC, 2 * V], BF16, tag="vT01")
        nc.sync.dma_start(vT01[:], vT01_d[:].rearrange("p (c w) -> p c w", w=2 * V))
        vT23 = persist.tile([128, KC, 2 * V], BF16, tag="vT23")
        nc.sync.dma_start(vT23[:], vT23_d[:].rearrange("p (c w) -> p c w", w=2 * V))
        wl_r = persist.tile([128, KC, KC, 128], BF16, tag="wlr")
        for m in range(KC):
            nc.sync.dma_start(wl_r[:, m],
                              wl_d[m].rearrange("p (c w) -> p c w", w=128))
        # videos natural + two ones columns: [v, (b, d+2)]
        vbr = persist.tile([128, BPC, D + 2], BF16, tag="vbr")
        nc.sync.dma_start(vbr[:], vones_d[:].rearrange("p (b w) -> p b w", w=D + 2))
        lT = [persist.tile([128, KC, L], BF16, tag=f"lT{b}", name=f"lT{b}")
              for b in range(BPC)]
        nc.sync.dma_start(lT[2][:], linesT_d[2].rearrange("p (c w) -> p c w", w=L))
        # tiny side load on the GpSimd queue (stores use it much later)
        nc.gpsimd.dma_start(maskb[:], maskb_d[:])

        # Q-B (Scalar) lines are delayed so Q-A's u-critical stream gets the
        # full DMA bandwidth at kernel start (bandwidth is shared per-packet).
        with tc.tile_wait_until(0.0045):
            for b in (0, 1, 3):
                nc.scalar.dma_start(lT[b][:],
                                    linesT_d[b].rearrange("p (c w) -> p c w", w=L))

        vs = {0: vT01, 1: vT23}
        u01 = persist.tile([128, KC, 2 * V], BF16, tag="u01")
        u23 = persist.tile([128, KC, 2 * V], BF16, tag="u23")
        us = {0: u01, 1: u23}

        # drain-engine rotation (PSUM->SBUF; GpSimd cannot access PSUM)
        dr = [lambda o, i: nc.scalar.copy(o, i),
              lambda o, i: nc.vector.tensor_copy(o, i)]

        # --- PE warmup: ramp the clock while DMAs stream --------------------
        for i in range(N_WARM):
            pw = pp_st.tile([128, 512], F32, tag="st")
            nc.tensor.matmul(pw[:], wsrc[:, 0:128], wsrc[:],
                             start=True, stop=True)

        # --- u for both pairs (u drains never overlap the out drains) -------
        with tc.tile_pool(name="pp_u", bufs=2, space="PSUM") as pp_u:
            ndr = _u_pair(nc, 0, pp_u, vs, us, wl_r, dr, 0)
            _u_pair(nc, 1, pp_u, vs, us, wl_r, dr, ndr)

        # --- batches: scores -> exp -> out, software-pipelined --------------
        with tc.tile_pool(name="pp_o1", bufs=3, space="PSUM") as pp_o1, \
             tc.tile_pool(name="pp_o2", bufs=3, space="PSUM") as pp_o2:
            eTs = {}
            for b in range(BPC):
                eTs[b] = _scores(nc, b, us, lT, pp_st, etpool, maskb)
                if b > 0:
                    _out_batch(nc, b - 1, eTs.pop(b - 1), vbr, pp_o1, pp_o2,
                               rpool, outpool, out_d)
            _out_batch(nc, BPC - 1, eTs.pop(BPC - 1), vbr, pp_o1, pp_o2,
                       rpool, outpool, out_d)


_CACHE = {}


def _build():
    if "nc" in _CACHE:
        return _CACHE["nc"]
    nc = bacc.Bacc("TRN2", target_bir_lowering=False, debug=False,
                   num_devices=N_CORES)
    linesT_d = nc.dram_tensor("linesT", [BPC, 128, KC * L], BF16,
                              kind="ExternalInput").ap()
    vT01_d = nc.dram_tensor("vT01", [128, KC * 2 * V], BF16,
                            kind="ExternalInput").ap()
    vT23_d = nc.dram_tensor("vT23", [128, KC * 2 * V], BF16,
                            kind="ExternalInput").ap()
    vones_d = nc.dram_tensor("vones", [128, BPC * (D + 2)], BF16,
                             kind="ExternalInput").ap()
    maskb_d = nc.dram_tensor("maskb", [V, BPC], F32, kind="ExternalInput").ap()
    wl_d = nc.dram_tensor("wl", [KC, 128, KC * 128], BF16,
                          kind="ExternalInput").ap()
    out_d = nc.dram_tensor("out", [BPC, L, D], BF16, kind="ExternalOutput").ap()
    with tile.TileContext(nc) as tc:
        _body(tc, out_d, linesT_d, vT01_d, vT23_d, vones_d, maskb_d, wl_d)
    nc.compile()
    _CACHE["nc"] = nc
    return nc


def _in_maps(lines, videos, video_mask, w_q, w_k):
    w_q = np.asarray(w_q, dtype=np.float32)
    w_k = np.asarray(w_k, dtype=np.float32)
    video_mask = np.asarray(video_mask)
    scale = np.float64(D) ** -0.5
    # scores = lines @ (w_q.T @ w_k * scale) @ videos^T; device wants WL[d', d] = W[d, d']
    WL = (scale * (w_k.astype(np.float64).T @ w_q.astype(np.float64))
          ).astype(np.float32)
    mask_bias = np.where(np.asarray(video_mask) == 0,
                         np.float32(-50.0), np.float32(0.0)).astype(np.float32)
    import ml_dtypes
    bf16 = ml_dtypes.bfloat16
    videos = np.asarray(videos, dtype=np.float32)
    lines = np.asarray(lines, dtype=np.float32)
    # vbr layout [v, (b, d+2)] per core
    vones = np.concatenate(
        [videos, np.ones((B, V, 2), dtype=np.float32)], axis=2).astype(bf16)
    vones = vones.reshape(N_CORES, BPC, V, D + 2).transpose(0, 2, 1, 3)
    vones = np.ascontiguousarray(vones.reshape(N_CORES, V, BPC * (D + 2)))
    # lT layout [b][p=d%128, (c=d//128, l)] per core
    linesT = lines.transpose(0, 2, 1).astype(bf16)          # [B, D, L]
    linesT = linesT.reshape(B, KC, 128, L).transpose(0, 2, 1, 3)
    linesT = np.ascontiguousarray(linesT.reshape(N_CORES, BPC, 128, KC * L))
    # vT pair layout [p=d'%128, (c, bpair, v)] per core
    videosT = videos.transpose(0, 2, 1).astype(bf16)        # [B, D, V]
    videosT = videosT.reshape(N_CORES, BPC, KC, 128, V).transpose(0, 3, 2, 1, 4)
    # -> [cores, 128, KC, BPC, V]; split pairs
    vT01 = np.ascontiguousarray(
        videosT[:, :, :, 0:2, :].reshape(N_CORES, 128, KC * 2 * V))
    vT23 = np.ascontiguousarray(
        videosT[:, :, :, 2:4, :].reshape(N_CORES, 128, KC * 2 * V))
    # wl m-slice-contiguous: wl[m][p, (c, s)] = WL[c*128+p, m*128+s]
    WLh = np.ascontiguousarray(
        WL.astype(bf16).reshape(KC, 128, KC, 128)
        .transpose(2, 1, 0, 3).reshape(KC, 128, KC * 128))
    maps = []
    for c in range(N_CORES):
        sl = slice(c * BPC, (c + 1) * BPC)
        maps.append({
            "linesT": linesT[c],
            "vT01": vT01[c],
            "vT23": vT23[c],
            "vones": vones[c],
            "maskb": np.ascontiguousarray(mask_bias[sl].T),
            "wl": WLh,
        })
    return maps


def kernel(lines, videos, video_mask, w_q, w_k):
    nc = _build()
    maps = _in_maps(lines, videos, video_mask, w_q, w_k)
    res = run_bass_kernel_spmd(nc, maps, list(range(N_CORES)))
    out = np.concatenate([res.results[c]["out"] for c in range(N_CORES)], axis=0)
    return np.ascontiguousarray(out.astype(np.float32))


# revision 14
# speedup vs baseline: 1.0118x; 1.0118x over previous
"""CrossAttentionPool forward on 8 TRN2 NeuronCores.

Reference computation (per batch b):
    q = lines[b] @ w_q.T ; k = videos[b] @ w_k.T
    scores = (q @ k.T) * D**-0.5, masked where video_mask==0
    out = softmax(scores, axis=-1) @ videos[b]

Strategy (data-parallel over batch, 4 batches/core):
    scores = lines @ W @ videos^T with W = (w_q.T @ w_k) * scale folded on host.
    All operands bf16; output stored bf16 and widened to fp32 on host.

    Device pipeline (memory-roofline ~9.0MB @ ~390GB/s/core):
      - PE warmup matmuls ramp the DVFS p-state (0.65/1.2/2.4 GHz; max only
        after 3us CONTINUOUS busy) while inputs stream; the compute order
        (u-pair0 -> batches 0,1 -> u-pair1 -> batches 2,3) keeps the PE from
        ever idling on a late input, which would reset the clock to 1.2GHz.
      - Two input queues: Sync carries the u-critical stream (vT01, wl
        m-slices, vones, vT23, lT2); Scalar carries lines b0,b1,b3.
      - u[d,v] = sum_d' W[d,d'] videos[v,d']  (36 MMs/pair, N=256)
      - scores^T = u^T lines^T (6 MMs N=512/batch); exp is ONE 512-wide
        Scalar activation per batch with the mask as exp-bias (-50).
      - out rows + softmax denominator in the same matmuls via two ones
        columns appended to videos; per l-chunk: DVE reciprocal, scaled
        PSUM drains alternating Scalar/DVE (GpSimd cannot read PSUM),
        per-chunk contiguous bf16 stores issued from GpSimd.
    PSUM rings (1 bank per buf, 8 banks): st 2, pu 2, po1 2, po2 2.
"""
import numpy as np
from contextlib import ExitStack
import concourse.bacc as bacc
import concourse.tile as tile
from concourse import mybir
from concourse.bass_utils import run_bass_kernel_spmd

N_CORES = 8
B, L, V, D = 32, 512, 128, 768
BPC = B // N_CORES          # batches per core
KC = D // 128               # 6 contraction chunks
LC = L // 128               # 4 line chunks
F32 = mybir.dt.float32
BF16 = mybir.dt.bfloat16
N_WARM = 9                  # warmup matmuls (ap=512) to ramp the PE clock


def _u_pair(nc, pair, pp_u, vs, us, wl_r, dr, ndr, hooks=None):
    """u = W @ videos^T for one batch pair (N=256).

    hooks[m]() runs right after m's drain — used to issue the lines DMAs
    from Scalar only once the u stream is underway, so the critical Q-A
    stream has the full bandwidth at kernel start.
    """
    for m in range(KC):
        pu = pp_u.tile([128, 256], F32, tag="pu")
        for c in range(KC):
            nc.tensor.matmul(pu[:], wl_r[:, m, c], vs[pair][:, c],
                             start=(c == 0), stop=(c == KC - 1))
        dr[ndr % 2](us[pair][:, m], pu[:])
        ndr += 1
        if hooks and m in hooks:
            hooks[m]()
    return ndr


def _scores(nc, b, us, lT, pp_st, etpool, maskb):
    psT = pp_st.tile([128, 512], F32, tag="st")
    ub = us[b // 2]
    for m in range(KC):
        nc.tensor.matmul(psT[:], ub[:, m, (b % 2) * V:(b % 2 + 1) * V],
                         lT[b][:, m, :], start=(m == 0), stop=(m == KC - 1))
    eT = etpool.tile([128, 512], BF16)
    nc.scalar.activation(eT[:], psT[:], mybir.ActivationFunctionType.Exp,
                         bias=maskb[:, b:b + 1])
    return eT


def _out_batch(nc, b, eT, vbr_b, pp_o1, pp_o2, rpool, outpool, out_d):
    """out rows + denominators for one batch; drains split Scalar/DVE."""
    for i in range(LC):
        ech = eT[:, i * 128:(i + 1) * 128]
        po1 = pp_o1.tile([128, 512], F32, tag="po1")
        nc.tensor.matmul(po1[:], ech, vbr_b[:, 0:512], start=True, stop=True)
        po2 = pp_o2.tile([128, 258], F32, tag="po2")
        nc.tensor.matmul(po2[:], ech, vbr_b[:, 512:D + 2],
                         start=True, stop=True)
        rec = rpool.tile([128, 1], F32)
        nc.vector.reciprocal(rec[:], po2[:, 256:257])
        osb = outpool.tile([128, D], BF16)
        if i % 2 == 0:
            nc.scalar.mul(osb[:, 0:512], po1[:], rec[:])
            nc.vector.tensor_scalar_mul(osb[:, 512:D], po2[:, 0:256], rec[:])
        else:
            nc.vector.tensor_scalar_mul(osb[:, 0:512], po1[:], rec[:])
            nc.scalar.mul(osb[:, 512:D], po2[:, 0:256], rec[:])
        seng = nc.gpsimd if (b * LC + i) % 2 == 0 else nc.sync
        seng.dma_start(out_d[b, i * 128:(i + 1) * 128, :], osb[:])


def _body(tc, out_d, linesT_d, vT01_d, vT23_d, vones_d, maskb_d, wl_d):
    nc = tc.nc
    with ExitStack() as ctx:
        const = ctx.enter_context(tc.tile_pool(name="const", bufs=1))
        persist = ctx.enter_context(tc.tile_pool(name="persist", bufs=1))
        etpool = ctx.enter_context(tc.tile_pool(name="etp", bufs=2))
        outpool = ctx.enter_context(tc.tile_pool(name="osb", bufs=6))
        rpool = ctx.enter_context(tc.tile_pool(name="rp", bufs=4))

        pp_st = ctx.enter_context(tc.tile_pool(name="pp_st", bufs=2, space="PSUM"))

        # --- warmup operand (memset early on gpsimd; no DMA) ----------------
        wsrc = const.tile([128, 512], BF16)
        nc.gpsimd.memset(wsrc[:], 0.125)
        maskb = const.tile([128, BPC], F32)

        # --- input DMAs -----------------------------------------------------
        # Q-A (Sync): u-critical stream, strictly ordered, then vones, lT2
        vT01 = persist.tile([128, KC, 2 * V], BF16, tag="vT01")
        nc.sync.dma_start(vT01[:], vT01_d[:].rearrange("p (c w) -> p c w", w=2 * V))
        vT23 = persist.tile([128, KC, 2 * V], BF16, tag="vT23")
        nc.sync.dma_start(vT23[:], vT23_d[:].rearrange("p (c w) -> p c w", w=2 * V))
        wl_r = persist.tile([128, KC, KC, 128], BF16, tag="wlr")
        for m in range(KC):
            nc.sync.dma_start(wl_r[:, m],
                              wl_d[m].rearrange("p (c w) -> p c w", w=128))
        # videos natural + two ones columns, per-batch tiles: [v, d+2]
        vbr = [persist.tile([128, D + 2], BF16, tag=f"vbr{b}", name=f"vbr{b}")
               for b in range(BPC)]
        nc.sync.dma_start(vbr[0][:], vones_d[:, 0 * (D + 2):1 * (D + 2)])
        nc.sync.dma_start(vbr[1][:], vones_d[:, 1 * (D + 2):2 * (D + 2)])
        lT = [persist.tile([128, KC, L], BF16, tag=f"lT{b}", name=f"lT{b}")
              for b in range(BPC)]
        nc.sync.dma_start(lT[2][:], linesT_d[2].rearrange("p (c w) -> p c w", w=L))
        nc.sync.dma_start(vbr[2][:], vones_d[:, 2 * (D + 2):3 * (D + 2)])
        nc.sync.dma_start(vbr[3][:], vones_d[:, 3 * (D + 2):4 * (D + 2)])
        # tiny side load on the GpSimd queue (stores use it much later)
        nc.gpsimd.dma_start(maskb[:], maskb_d[:])

        # Q-B (Scalar) lines are delayed so Q-A's u-critical stream gets the
        # full DMA bandwidth at kernel start (bandwidth is shared per-packet).
        with tc.tile_wait_until(0.008):
            for b in (0, 1, 3):
                nc.scalar.dma_start(lT[b][:],
                                    linesT_d[b].rearrange("p (c w) -> p c w", w=L))

        vs = {0: vT01, 1: vT23}
        u01 = persist.tile([128, KC, 2 * V], BF16, tag="u01")
        u23 = persist.tile([128, KC, 2 * V], BF16, tag="u23")
        us = {0: u01, 1: u23}

        # drain-engine rotation (PSUM->SBUF; GpSimd cannot access PSUM)
        dr = [lambda o, i: nc.scalar.copy(o, i),
              lambda o, i: nc.vector.tensor_copy(o, i)]

        # --- PE warmup: ramp the clock while DMAs stream --------------------
        for i in range(N_WARM):
            pw = pp_st.tile([128, 512], F32, tag="st")
            nc.tensor.matmul(pw[:], wsrc[:, 0:128], wsrc[:],
                             start=True, stop=True)

        # --- emission: u0, s0, s1, o0, o1, u1, s2, o2, s3, o3 ---------------
        # The list scheduler runs whatever is ready: u-pair1 fills any stall
        # in the scores/out pipeline (late lines, drain backpressure), so the
        # PE never idles and the out drains start ~4us earlier.
        pp_u = ctx.enter_context(tc.tile_pool(name="pp_u", bufs=2, space="PSUM"))
        pp_o1 = ctx.enter_context(tc.tile_pool(name="pp_o1", bufs=2, space="PSUM"))
        pp_o2 = ctx.enter_context(tc.tile_pool(name="pp_o2", bufs=2, space="PSUM"))
        ndr = _u_pair(nc, 0, pp_u, vs, us, wl_r, dr, 0)
        eT0 = _scores(nc, 0, us, lT, pp_st, etpool, maskb)
        eT1 = _scores(nc, 1, us, lT, pp_st, etpool, maskb)
        _out_batch(nc, 0, eT0, vbr[0], pp_o1, pp_o2, rpool, outpool, out_d)
        _out_batch(nc, 1, eT1, vbr[1], pp_o1, pp_o2, rpool, outpool, out_d)
        _u_pair(nc, 1, pp_u, vs, us, wl_r, dr, ndr)
        eT2 = _scores(nc, 2, us, lT, pp_st, etpool, maskb)
        _out_batch(nc, 2, eT2, vbr[2], pp_o1, pp_o2, rpool, outpool, out_d)
        eT3 = _scores(nc, 3, us, lT, pp_st, etpool, maskb)
        _out_batch(nc, 3, eT3, vbr[3], pp_o1, pp_o2, rpool, outpool, out_d)


_CACHE = {}


def _build():
    if "nc" in _CACHE:
        return _CACHE["nc"]
    nc = bacc.Bacc("TRN2", target_bir_lowering=False, debug=False,
                   num_devices=N_CORES)
    linesT_d = nc.dram_tensor("linesT", [BPC, 128, KC * L], BF16,
                              kind="ExternalInput").ap()
    vT01_d = nc.dram_tensor("vT01", [128, KC * 2 * V], BF16,
                            kind="ExternalInput").ap()
    vT23_d = nc.dram_tensor("vT23", [128, KC * 2 * V], BF16,
                            kind="ExternalInput").ap()
    vones_d = nc.dram_tensor("vones", [128, BPC * (D + 2)], BF16,
                             kind="ExternalInput").ap()
    maskb_d = nc.dram_tensor("maskb", [V, BPC], F32, kind="ExternalInput").ap()
    wl_d = nc.dram_tensor("wl", [KC, 128, KC * 128], BF16,
                          kind="ExternalInput").ap()
    out_d = nc.dram_tensor("out", [BPC, L, D], BF16, kind="ExternalOutput").ap()
    with tile.TileContext(nc) as tc:
        _body(tc, out_d, linesT_d, vT01_d, vT23_d, vones_d, maskb_d, wl_d)
    nc.compile()
    _CACHE["nc"] = nc
    return nc


def _in_maps(lines, videos, video_mask, w_q, w_k):
    w_q = np.asarray(w_q, dtype=np.float32)
    w_k = np.asarray(w_k, dtype=np.float32)
    video_mask = np.asarray(video_mask)
    scale = np.float64(D) ** -0.5
    # scores = lines @ (w_q.T @ w_k * scale) @ videos^T; device wants WL[d', d] = W[d, d']
    WL = (scale * (w_k.astype(np.float64).T @ w_q.astype(np.float64))
          ).astype(np.float32)
    mask_bias = np.where(np.asarray(video_mask) == 0,
                         np.float32(-50.0), np.float32(0.0)).astype(np.float32)
    import ml_dtypes
    bf16 = ml_dtypes.bfloat16
    videos = np.asarray(videos, dtype=np.float32)
    lines = np.asarray(lines, dtype=np.float32)
    # vbr layout [v, (b, d+2)] per core
    vones = np.concatenate(
        [videos, np.ones((B, V, 2), dtype=np.float32)], axis=2).astype(bf16)
    vones = vones.reshape(N_CORES, BPC, V, D + 2).transpose(0, 2, 1, 3)
    vones = np.ascontiguousarray(vones.reshape(N_CORES, V, BPC * (D + 2)))
    # lT layout [b][p=d%128, (c=d//128, l)] per core
    linesT = lines.transpose(0, 2, 1).astype(bf16)          # [B, D, L]
    linesT = linesT.reshape(B, KC, 128, L).transpose(0, 2, 1, 3)
    linesT = np.ascontiguousarray(linesT.reshape(N_CORES, BPC, 128, KC * L))
    # vT pair layout [p=d'%128, (c, bpair, v)] per core
    videosT = videos.transpose(0, 2, 1).astype(bf16)        # [B, D, V]
    videosT = videosT.reshape(N_CORES, BPC, KC, 128, V).transpose(0, 3, 2, 1, 4)
    # -> [cores, 128, KC, BPC, V]; split pairs
    vT01 = np.ascontiguousarray(
        videosT[:, :, :, 0:2, :].reshape(N_CORES, 128, KC * 2 * V))
    vT23 = np.ascontiguousarray(
        videosT[:, :, :, 2:4, :].reshape(N_CORES, 128, KC * 2 * V))
    # wl m-slice-contiguous: wl[m][p, (c, s)] = WL[c*128+p, m*128+s]
    WLh = np.ascontiguousarray(
        WL.astype(bf16).reshape(KC, 128, KC, 128)
        .transpose(2, 1, 0, 3).reshape(KC, 128, KC * 128))
    maps = []
    for c in range(N_CORES):
        sl = slice(c * BPC, (c + 1) * BPC)
        maps.append({
            "linesT": linesT[c],
            "vT01": vT01[c],
            "vT23": vT23[c],
            "vones": vones[c],
            "maskb": np.ascontiguousarray(mask_bias[sl].T),
            "wl": WLh,
        })
    return maps


def kernel(lines, videos, video_mask, w_q, w_k):
    nc = _build()
    maps = _in_maps(lines, videos, video_mask, w_q, w_k)
    res = run_bass_kernel_spmd(nc, maps, list(range(N_CORES)))
    out = np.concatenate([res.results[c]["out"] for c in range(N_CORES)], axis=0)
    return np.ascontiguousarray(out.astype(np.float32))
